# revision 9
# baseline (speedup 1.0000x reference)
"""Trainium2 Bass kernel for grouped per-atom MLPs (AtomicNN energy eval).

Math: e[s, a] = W3[a].T tanh(W2[a].T tanh(W1[a].T g[s,a] + b1[a]) + b2[a]) + b3[a]
Shapes: g [4096, 1024, 5], per-atom MLP 5 -> 64 -> 64 -> 1.

Strategy (8 NeuronCores, SPMD):
 - Shard the atom axis: core c owns atoms [128c, 128c+128). All 4096 structs
   stream through each core (expert-style parallelism; weights are small and
   unique per atom, so this avoids replicating the weight DMA 8x).
 - Atoms are processed in pairs (2x64 = 128 PE rows/cols). The struct axis is
   the matmul moving (N) dimension, 512 columns per matmul (one PSUM bank).
 - Layer 1: lhsT = blockdiag(W1[2p], W1[2p+1]) with an extra bias row
   ([11, 128]); rhs = transposed g pair tile + ones row ([11, 512]) -> fuses
   the b1 add into the matmul.
 - Layer 2: lhsT = blockdiag(W2[2p], W2[2p+1]) ([128, 128]).
 - Layer 3: 64 accumulating matmuls (one per pair) into a single PSUM bank:
   lhsT[:, 2p] = [W3[2p]; 0], lhsT[:, 2p+1] = [0; W3[2p+1]] -> builds the
   [128 atoms, 512 structs] transposed-output block directly.
 - tanh on the Scalar (ACT) engine, batched over multiple PSUM banks per
   instruction (4 pairs for layer 1, 2 pairs for layer 2) to amortize the
   per-instruction overhead; ACT is the bottleneck engine for this problem.
 - Matmul dtype float32r (~1.5e-4 rel err vs fp32; same net speed as bf16
   here since the PE runs throttled at 1.2 GHz under this dependency mix).
 - g is pre-transposed host-side to [chunk, 11, pair, 512] so all device DMAs
   are contiguous-2KB-row strided loads.
"""

from contextlib import ExitStack

import numpy as np

S, A, D, H = 4096, 1024, 5, 64
NCORES = 8
ACORE = A // NCORES  # 128 atoms per core
NPAIR = ACORE // 2  # 64 atom pairs per core
NS = 512  # struct chunk = one PSUM bank of fp32
NCHUNK = S // NS  # 8
KG = D * 2 + 1  # 11: two atoms' descriptors + ones row for the b1 fold
G1 = 2  # pairs per layer-1 tanh batch (2 PSUM banks, double-buffered)
G2 = 2  # pairs per layer-2 tanh batch (2 PSUM banks)

_compiled = {}

MM_DT = "bfloat16"  # matmul operand dtype: bfloat16 | float32r
NWU = 24  # warm-up matmuls (bridge the HAM un-throttle into the main stream)


def _build(with_b2):
    import concourse.tile as tile
    import concourse.mybir as mybir
    from concourse import bacc

    dt = mybir.dt
    mdt = getattr(dt, MM_DT)
    Tanh = mybir.ActivationFunctionType.Tanh

    nc = bacc.Bacc(
        "TRN2", target_bir_lowering=False, debug=False, num_devices=NCORES
    )
    gt = nc.declare_dram_parameter(
        "gt", [NCHUNK, KG, NPAIR, NS], mdt, isOutput=False
    )
    w1 = nc.declare_dram_parameter(
        "w1", [KG, NPAIR * 128], mdt, isOutput=False
    )
    w2 = nc.declare_dram_parameter(
        "w2", [128, NPAIR * 128], mdt, isOutput=False
    )
    w3 = nc.declare_dram_parameter(
        "w3", [128, NPAIR * 128], mdt, isOutput=False
    )
    if with_b2:
        b2d = nc.declare_dram_parameter("b2d", [128, NPAIR], dt.float32, isOutput=False)
    b3d = nc.declare_dram_parameter("b3d", [128, 1], dt.float32, isOutput=False)
    eo = nc.declare_dram_parameter("eo", [128, S], dt.float32, isOutput=True)

    with tile.TileContext(nc) as tc, ExitStack() as ctx:
        wp = ctx.enter_context(tc.tile_pool(name="wp", bufs=1))
        gp = ctx.enter_context(tc.tile_pool(name="gp", bufs=2))
        h1p = ctx.enter_context(tc.tile_pool(name="h1p", bufs=3))
        h2p = ctx.enter_context(tc.tile_pool(name="h2p", bufs=3))
        eop = ctx.enter_context(tc.tile_pool(name="eop", bufs=2))
        z1p = ctx.enter_context(tc.tile_pool(name="z1p", bufs=2, space="PSUM"))
        z2p = ctx.enter_context(tc.tile_pool(name="z2p", bufs=1, space="PSUM"))
        etp = ctx.enter_context(tc.tile_pool(name="etp", bufs=2, space="PSUM"))

        # w1 first on the sync queue (it is small and unblocks the warm-up
        # burst + layer-1); the big w2/w3 go on their own queues so they are
        # ready before the stream reaches layer 2/3.
        w1t = wp.tile([KG, NPAIR * 128], mdt)
        nc.sync.dma_start(w1t[:], w1[:])
        # Small early slice of w2 for the warm-up burst: the HAM watches PE
        # array *activity*, so warm-up matmuls must be K=128 (full array);
        # K=11 bursts (w1-shaped) never trip the un-throttle window.
        wwut = wp.tile([128, 640], mdt)
        nc.sync.dma_start(wwut[:], w2[:, 0:640])
        w2t = wp.tile([128, NPAIR * 128], mdt)
        nc.gpsimd.dma_start(w2t[:], w2[:])
        w3t = wp.tile([128, NPAIR * 128], mdt)
        nc.scalar.dma_start(w3t[:], w3[:])
        b3t = wp.tile([128, 1], dt.float32)
        nc.scalar.dma_start(b3t[:], b3d[:])
        if with_b2:
            b2t = wp.tile([128, NPAIR], dt.float32)
            nc.scalar.dma_start(b2t[:], b2d[:])

        NGRP = NPAIR // G1  # groups per chunk
        et_tiles = {}

        HALF = NPAIR // 4  # pairs per staged g DMA (quarter chunk)
        NHALF = NCHUNK * 4
        gstage = {}

        def ensure_half(hq):
            if hq in gstage or hq >= NHALF:
                return
            hc, hi = divmod(hq, 4)
            gs = gp.tile([KG, HALF * NS], mdt, name=f"gs{hq}", tag="gs")
            p0 = hi * HALF
            nc.sync.dma_start(gs[:], gt[hc, :, p0 : p0 + HALF, :])
            gstage[hq] = gs

        def stage_front(c, g):
            """Layer-1 matmuls + tanh1 from the staged g (prefetch one half
            ahead; the gs pool double-buffers)."""
            hq = c * 4 + (g * G1) // HALF
            ensure_half(hq)
            ensure_half(hq + 1)
            gs = gstage[hq]
            half = (c, (g * G1) // HALF)
            z1 = z1p.tile([128, G1 * NS], dt.float32)
            for i in range(G1):
                p = g * G1 + i
                off = (p - half[1] * HALF) * NS
                nc.tensor.matmul(
                    z1[:, i * NS : (i + 1) * NS],
                    w1t[:, p * 128 : (p + 1) * 128],
                    gs[:, off : off + NS],
                    start=True,
                    stop=True,
                )
            h1 = h1p.tile([128, G1 * NS], mdt)
            nc.scalar.activation(h1[:], z1[:], Tanh)
            return h1

        def stage_back(c, g, h1):
            """Layer-2 matmuls (batched z2), tanh2, layer-3 accumulation;
            flush et at chunk end."""
            if c not in et_tiles:
                et_tiles[c] = etp.tile([128, NS], dt.float32, name=f"et{c}", tag="et")
            et = et_tiles[c]
            for j in range(G1 // G2):
                z2 = z2p.tile([128, G2 * NS], dt.float32, name=f"z2_{c}_{g}_{j}", tag="z2")
                for k in range(G2):
                    p = g * G1 + j * G2 + k
                    q = j * G2 + k
                    nc.tensor.matmul(
                        z2[:, k * NS : (k + 1) * NS],
                        w2t[:, p * 128 : (p + 1) * 128],
                        h1[:, q * NS : (q + 1) * NS],
                        start=True,
                        stop=True,
                    )
                if with_b2:
                    for k in range(G2):
                        p = g * G1 + j * G2 + k
                        nc.vector.tensor_scalar_add(
                            z2[:, k * NS : (k + 1) * NS],
                            z2[:, k * NS : (k + 1) * NS],
                            b2t[:, p : p + 1],
                        )
                h2 = h2p.tile([128, G2 * NS], mdt, name=f"h2_{c}_{g}_{j}", tag="h2")
                nc.scalar.activation(h2[:], z2[:], Tanh)
                for k in range(G2):
                    p = g * G1 + j * G2 + k
                    nc.tensor.matmul(
                        et[:],
                        w3t[:, p * 128 : (p + 1) * 128],
                        h2[:, k * NS : (k + 1) * NS],
                        start=(p == 0),
                        stop=(p == NPAIR - 1),
                    )
            if g == NGRP - 1:
                eot = eop.tile([128, NS], dt.float32)
                nc.vector.tensor_scalar_add(eot[:], et[:], b3t[:])
                nc.sync.dma_start(eo[:, c * NS : (c + 1) * NS], eot[:])
                del et_tiles[c]

        # PE warm-up: a dense matmul burst anchored on the small w1t DMA
        # (ready ~10us in, vs ~24us for w2t), long enough to bridge into the
        # first real matmuls with no idle gap -- the HAM un-throttles to
        # 2.4 GHz during the burst, and as long as the main stream never
        # leaves a multi-us PE idle gap it stays warm for the whole kernel.
        # Full-array (K=128) warm-up matmuls, rotating output banks so fill
        # overlaps drain: streams gapless and trips the HAM un-throttle
        # window ~3.4us in; the main stream's K=128 L2/L3 matmuls then keep
        # the activity high enough that it never re-throttles.
        zwua = z1p.tile([128, G1 * NS], dt.float32, name="zwua", tag="z1")
        zwub = z1p.tile([128, G1 * NS], dt.float32, name="zwub", tag="z1")
        for i in range(NWU):
            zt = zwua if (i % 4) < 2 else zwub
            c0 = (i % 2) * NS
            nc.tensor.matmul(
                zt[:, c0 : c0 + NS],
                wwut[:, 0:128],
                wwut[:, 128:640],
                start=True,
                stop=True,
            )

        # Software pipeline: issue group q's front stage before group q-1's
        # back stage so the ACT engine always has an independent tanh queued.
        pending = None
        for q in range(NCHUNK * NGRP):
            c, g = divmod(q, NGRP)
            h1 = stage_front(c, g)
            if pending is not None:
                stage_back(*pending)
            pending = (c, g, h1)
        stage_back(*pending)
    nc.compile()
    return nc


def _prep_core(c, g, W1, b1, W2, b2, W3, b3, with_b2):
    import ml_dtypes

    at = slice(c * ACORE, (c + 1) * ACORE)
    f32 = np.float32
    mdt = ml_dtypes.bfloat16 if MM_DT == "bfloat16" else np.float32

    # gt: [NCHUNK, 11, NPAIR, NS]; row r<10: descriptor d=r%5 of even/odd atom
    # of each pair; row 10: ones (streams the b1 fold).
    gc = g[:, at, :]  # [S, 128, 5]
    gT = np.ascontiguousarray(gc.transpose(1, 2, 0))  # [128, 5, S]
    gT = gT.reshape(NPAIR, 2 * D, S)  # [64, 10, S]
    gt = np.empty((NCHUNK, KG, NPAIR, NS), dtype=f32)
    # [64, 10, S] -> [10, 64, NCHUNK, NS] -> chunk-major
    gt[:, : 2 * D] = gT.transpose(1, 0, 2).reshape(2 * D, NPAIR, NCHUNK, NS).transpose(2, 0, 1, 3)
    gt[:, 2 * D] = 1.0

    W1c, b1c = W1[at], b1[at]  # [128, 5, 64], [128, 64]
    w1a = np.zeros((NPAIR, KG, 128), dtype=f32)
    w1a[:, :D, :H] = W1c[0::2]
    w1a[:, D : 2 * D, H:] = W1c[1::2]
    w1a[:, 2 * D, :H] = b1c[0::2]
    w1a[:, 2 * D, H:] = b1c[1::2]
    w1d = np.ascontiguousarray(w1a.transpose(1, 0, 2)).reshape(KG, NPAIR * 128)

    W2c = W2[at]  # [128, 64, 64]
    w2a = np.zeros((NPAIR, 128, 128), dtype=f32)
    w2a[:, :H, :H] = W2c[0::2]
    w2a[:, H:, H:] = W2c[1::2]
    w2d = np.ascontiguousarray(w2a.transpose(1, 0, 2)).reshape(128, NPAIR * 128)

    W3c = W3[at][..., 0]  # [128, 64]
    w3a = np.zeros((NPAIR, 128, 128), dtype=f32)
    for p in range(NPAIR):
        w3a[p, :H, 2 * p] = W3c[2 * p]
        w3a[p, H:, 2 * p + 1] = W3c[2 * p + 1]
    w3d = np.ascontiguousarray(w3a.transpose(1, 0, 2)).reshape(128, NPAIR * 128)

    in_map = {
        "gt": np.ascontiguousarray(gt).astype(mdt),
        "w1": w1d.astype(mdt),
        "w2": w2d.astype(mdt),
        "w3": w3d.astype(mdt),
        "b3d": np.ascontiguousarray(b3[at]).astype(f32),
    }
    if with_b2:
        b2c = b2[at]  # [128, 64]
        in_map["b2d"] = np.ascontiguousarray(
            np.concatenate([b2c[0::2].T, b2c[1::2].T], axis=0)
        ).astype(f32)
    return in_map


def kernel(g, W1, b1, W2, b2, W3, b3):
    from concourse.bass_utils import run_bass_kernel_spmd

    g = np.asarray(g, dtype=np.float32)
    W1 = np.asarray(W1, dtype=np.float32)
    b1 = np.asarray(b1, dtype=np.float32)
    W2 = np.asarray(W2, dtype=np.float32)
    b2 = np.asarray(b2, dtype=np.float32)
    W3 = np.asarray(W3, dtype=np.float32)
    b3 = np.asarray(b3, dtype=np.float32)

    with_b2 = bool(np.any(b2))
    if with_b2 not in _compiled:
        _compiled[with_b2] = _build(with_b2)
    nc = _compiled[with_b2]

    in_maps = [
        _prep_core(c, g, W1, b1, W2, b2, W3, b3, with_b2) for c in range(NCORES)
    ]
    res = run_bass_kernel_spmd(nc, in_maps, list(range(NCORES)))

    e = np.empty((S, A), dtype=np.float32)
    for c in range(NCORES):
        e[:, c * ACORE : (c + 1) * ACORE] = res.results[c]["eo"].T
    return e



# revision 11
# speedup vs baseline: 1.1241x; 1.1241x over previous
"""Trainium2 Bass kernel for grouped per-atom MLPs (AtomicNN energy eval).

Math: e[s, a] = W3[a].T tanh(W2[a].T tanh(W1[a].T g[s,a] + b1[a]) + b2[a]) + b3[a]
Shapes: g [4096, 1024, 5], per-atom MLP 5 -> 64 -> 64 -> 1.

Strategy (8 NeuronCores, SPMD):
 - Shard the atom axis: core c owns atoms [128c, 128c+128). All 4096 structs
   stream through each core (expert-style parallelism; weights are small and
   unique per atom, so this avoids replicating the weight DMA 8x).
 - Atoms are processed in pairs (2x64 = 128 PE rows/cols). The struct axis is
   the matmul moving (N) dimension, 512 columns per matmul (one PSUM bank).
 - Layer 1: lhsT = blockdiag(W1[2p], W1[2p+1]) + a bias row, zero-padded to
   [128, 128]; rhs = transposed g pair tile + ones row in rows 0..10 of a
   [128, *] SBUF tile whose rows 11..127 are memset to zero once. K=128
   costs nothing on the PE (time is moving-dim cycles) but keeps the HAM
   activity monitor high -- K=11 matmuls read as a ~9%-active array and the
   HAM re-throttles the PE clock to 1.2 GHz.
 - Layer 2: lhsT = blockdiag(W2[2p], W2[2p+1]) ([128, 128]).
 - Layer 3: 64 accumulating matmuls (one per pair) into a single PSUM bank:
   lhsT[:, 2p] = [W3[2p]; 0], lhsT[:, 2p+1] = [0; W3[2p+1]] -> builds the
   [128 atoms, 512 structs] transposed-output block directly.
 - tanh on the Scalar (ACT) engine (the bottleneck once the PE runs warm),
   batched 2 pairs / 2 PSUM banks per instruction.
 - Matmul dtype bfloat16 (~6e-3 rel err): halves DMA and enables overlapped
   (hidden) LDWEIGHTS, unlike float32r.
 - PE warm-up: a gapless K=128 burst right after the w1 DMA trips the HAM
   un-throttle (1.2 -> 2.4 GHz) before the main stream starts; output banks
   rotate so fill overlaps drain (same-bank rewrites serialize).
 - g is pre-transposed host-side to [chunk, 11, pair, 512] so all device DMAs
   are contiguous-2KB-row strided loads.
"""

from contextlib import ExitStack

import numpy as np

S, A, D, H = 4096, 1024, 5, 64
NCORES = 8
ACORE = A // NCORES  # 128 atoms per core
NPAIR = ACORE // 2  # 64 atom pairs per core
NS = 512  # struct chunk = one PSUM bank of fp32
NCHUNK = S // NS  # 8
KG = D * 2 + 1  # 11: two atoms' descriptors + ones row for the b1 fold
G1 = 2  # pairs per layer-1 tanh batch (2 PSUM banks, double-buffered)
G2 = 2  # pairs per layer-2 tanh batch (2 PSUM banks)

_compiled = {}

MM_DT = "bfloat16"  # matmul operand dtype: bfloat16 | float32r
NWU = 40  # warm-up matmuls (bridge the HAM un-throttle into the main stream)


def _build(with_b2):
    import concourse.tile as tile
    import concourse.mybir as mybir
    from concourse import bacc

    dt = mybir.dt
    mdt = getattr(dt, MM_DT)
    Tanh = mybir.ActivationFunctionType.Tanh

    nc = bacc.Bacc(
        "TRN2", target_bir_lowering=False, debug=False, num_devices=NCORES
    )
    gt = nc.declare_dram_parameter(
        "gt", [NCHUNK, KG, NPAIR, NS], mdt, isOutput=False
    )
    w1 = nc.declare_dram_parameter(
        "w1", [128, NPAIR * 128], mdt, isOutput=False
    )
    w2 = nc.declare_dram_parameter(
        "w2", [128, NPAIR * 128], mdt, isOutput=False
    )
    w3 = nc.declare_dram_parameter(
        "w3", [128, NPAIR * 128], mdt, isOutput=False
    )
    if with_b2:
        b2d = nc.declare_dram_parameter("b2d", [128, NPAIR], dt.float32, isOutput=False)
    b3d = nc.declare_dram_parameter("b3d", [128, 1], dt.float32, isOutput=False)
    eo = nc.declare_dram_parameter("eo", [128, S], dt.float32, isOutput=True)

    HALF = NPAIR // 4  # pairs per staged g DMA (quarter chunk)
    NHALF = NCHUNK * 4

    with tile.TileContext(nc) as tc, ExitStack() as ctx:
        wp = ctx.enter_context(tc.tile_pool(name="wp", bufs=1))
        h1p = ctx.enter_context(tc.tile_pool(name="h1p", bufs=3))
        h2p = ctx.enter_context(tc.tile_pool(name="h2p", bufs=3))
        eop = ctx.enter_context(tc.tile_pool(name="eop", bufs=2))
        z1p = ctx.enter_context(tc.tile_pool(name="z1p", bufs=2, space="PSUM"))
        z2p = ctx.enter_context(tc.tile_pool(name="z2p", bufs=1, space="PSUM"))
        etp = ctx.enter_context(tc.tile_pool(name="etp", bufs=2, space="PSUM"))

        # w1 first on its queue: it is what the warm-up burst and layer 1
        # need. w2/w3 are split in halves so layer 2/3 weights land before a
        # warm-clock stream reaches them. The sync queue carries only g.
        w1t = wp.tile([128, NPAIR * 128], mdt)
        nc.gpsimd.dma_start(w1t[:], w1[:])
        w2t = wp.tile([128, NPAIR * 128], mdt)
        nc.gpsimd.dma_start(w2t[:, : NPAIR * 64], w2[:, : NPAIR * 64])
        nc.gpsimd.dma_start(w2t[:, NPAIR * 64 :], w2[:, NPAIR * 64 :])
        w3t = wp.tile([128, NPAIR * 128], mdt)
        nc.scalar.dma_start(w3t[:, : NPAIR * 64], w3[:, : NPAIR * 64])
        nc.scalar.dma_start(w3t[:, NPAIR * 64 :], w3[:, NPAIR * 64 :])
        b3t = wp.tile([128, 1], dt.float32)
        nc.scalar.dma_start(b3t[:], b3d[:])
        if with_b2:
            b2t = wp.tile([128, NPAIR], dt.float32)
            nc.scalar.dma_start(b2t[:], b2d[:])

        # Two persistent g staging buffers (manual double-buffer). Rows
        # 0..10 are rewritten by each half-chunk DMA; rows 11..127 are
        # zeroed once so the K=128 layer-1 matmul contracts against real
        # zeros (w1 pad rows are zero too, but 0*garbage could be NaN).
        gsA = wp.tile([128, HALF * NS], mdt)
        gsB = wp.tile([128, HALF * NS], mdt)
        nc.vector.memset(gsA[:], 0)
        nc.vector.memset(gsB[:], 0)

        gstage = {}

        def ensure_half(hq):
            if hq in gstage or hq >= NHALF:
                return
            hc, hi = divmod(hq, 4)
            gs = gsA if hq % 2 == 0 else gsB
            p0 = hi * HALF
            nc.sync.dma_start(gs[:KG, :], gt[hc, :, p0 : p0 + HALF, :])
            gstage[hq] = gs

        NGRP = NPAIR // G1  # groups per chunk
        et_tiles = {}

        def stage_front(c, g):
            """Layer-1 matmuls + tanh1 from the staged g (prefetch one half
            ahead; gsA/gsB double-buffer)."""
            hq = c * 4 + (g * G1) // HALF
            ensure_half(hq)
            ensure_half(hq + 1)
            gs = gstage[hq]
            half = (c, (g * G1) // HALF)
            z1 = z1p.tile([128, G1 * NS], dt.float32)
            for i in range(G1):
                p = g * G1 + i
                off = (p - half[1] * HALF) * NS
                nc.tensor.matmul(
                    z1[:, i * NS : (i + 1) * NS],
                    w1t[:, p * 128 : (p + 1) * 128],
                    gs[:, off : off + NS],
                    start=True,
                    stop=True,
                )
            h1 = h1p.tile([128, G1 * NS], mdt)
            nc.scalar.activation(h1[:], z1[:], Tanh)
            return h1

        def stage_back(c, g, h1):
            """Layer-2 matmuls (batched z2), tanh2, layer-3 accumulation;
            flush et at chunk end."""
            if c not in et_tiles:
                et_tiles[c] = etp.tile([128, NS], dt.float32, name=f"et{c}", tag="et")
            et = et_tiles[c]
            for j in range(G1 // G2):
                z2 = z2p.tile([128, G2 * NS], dt.float32, name=f"z2_{c}_{g}_{j}", tag="z2")
                for k in range(G2):
                    p = g * G1 + j * G2 + k
                    q = j * G2 + k
                    nc.tensor.matmul(
                        z2[:, k * NS : (k + 1) * NS],
                        w2t[:, p * 128 : (p + 1) * 128],
                        h1[:, q * NS : (q + 1) * NS],
                        start=True,
                        stop=True,
                    )
                if with_b2:
                    for k in range(G2):
                        p = g * G1 + j * G2 + k
                        nc.vector.tensor_scalar_add(
                            z2[:, k * NS : (k + 1) * NS],
                            z2[:, k * NS : (k + 1) * NS],
                            b2t[:, p : p + 1],
                        )
                h2 = h2p.tile([128, G2 * NS], mdt, name=f"h2_{c}_{g}_{j}", tag="h2")
                nc.scalar.activation(h2[:], z2[:], Tanh)
                for k in range(G2):
                    p = g * G1 + j * G2 + k
                    nc.tensor.matmul(
                        et[:],
                        w3t[:, p * 128 : (p + 1) * 128],
                        h2[:, k * NS : (k + 1) * NS],
                        start=(p == 0),
                        stop=(p == NPAIR - 1),
                    )
            if g == NGRP - 1:
                eot = eop.tile([128, NS], dt.float32)
                nc.vector.tensor_scalar_add(eot[:], et[:], b3t[:])
                nc.sync.dma_start(eo[:, c * NS : (c + 1) * NS], eot[:])
                del et_tiles[c]

        # PE warm-up: gapless K=128 matmul burst anchored on the w1t DMA.
        # The HAM un-throttles the PE to 2.4 GHz after one fully-busy
        # 4096-cycle window; the burst bridges into the main stream (which
        # is all K=128 matmuls) so activity never drops low enough to
        # re-throttle. Output banks rotate: same-bank rewrites serialize
        # fill-after-drain and would leave ~100ns/matmul idle.
        zwua = z1p.tile([128, G1 * NS], dt.float32, name="zwua", tag="z1")
        zwub = z1p.tile([128, G1 * NS], dt.float32, name="zwub", tag="z1")
        for i in range(NWU):
            zt = zwua if (i % 4) < 2 else zwub
            c0 = (i % 2) * NS
            nc.tensor.matmul(
                zt[:, c0 : c0 + NS],
                w1t[:, (i % 16) * 128 : (i % 16) * 128 + 128],
                w1t[:, 2048 : 2048 + NS],
                start=True,
                stop=True,
            )

        # Software pipeline: issue group q's front stage before group q-1's
        # back stage so the ACT engine always has an independent tanh queued.
        pending = None
        for q in range(NCHUNK * NGRP):
            c, g = divmod(q, NGRP)
            h1 = stage_front(c, g)
            if pending is not None:
                stage_back(*pending)
            pending = (c, g, h1)
        stage_back(*pending)
    nc.compile()
    return nc


def _prep_core(c, g, W1, b1, W2, b2, W3, b3, with_b2):
    import ml_dtypes

    at = slice(c * ACORE, (c + 1) * ACORE)
    f32 = np.float32
    mdt = ml_dtypes.bfloat16 if MM_DT == "bfloat16" else np.float32

    # gt: [NCHUNK, 11, NPAIR, NS]; row r<10: descriptor d=r%5 of even/odd atom
    # of each pair; row 10: ones (streams the b1 fold).
    gc = g[:, at, :]  # [S, 128, 5]
    gT = np.ascontiguousarray(gc.transpose(1, 2, 0))  # [128, 5, S]
    gT = gT.reshape(NPAIR, 2 * D, S)  # [64, 10, S]
    gt = np.empty((NCHUNK, KG, NPAIR, NS), dtype=f32)
    # [64, 10, S] -> [10, 64, NCHUNK, NS] -> chunk-major
    gt[:, : 2 * D] = gT.transpose(1, 0, 2).reshape(2 * D, NPAIR, NCHUNK, NS).transpose(2, 0, 1, 3)
    gt[:, 2 * D] = 1.0

    W1c, b1c = W1[at], b1[at]  # [128, 5, 64], [128, 64]
    # Zero-padded to K=128 so layer-1 matmuls use the full PE array (keeps
    # the HAM activity monitor from re-throttling the PE clock).
    w1a = np.zeros((NPAIR, 128, 128), dtype=f32)
    w1a[:, :D, :H] = W1c[0::2]
    w1a[:, D : 2 * D, H:] = W1c[1::2]
    w1a[:, 2 * D, :H] = b1c[0::2]
    w1a[:, 2 * D, H:] = b1c[1::2]
    w1d = np.ascontiguousarray(w1a.transpose(1, 0, 2)).reshape(128, NPAIR * 128)

    W2c = W2[at]  # [128, 64, 64]
    w2a = np.zeros((NPAIR, 128, 128), dtype=f32)
    w2a[:, :H, :H] = W2c[0::2]
    w2a[:, H:, H:] = W2c[1::2]
    w2d = np.ascontiguousarray(w2a.transpose(1, 0, 2)).reshape(128, NPAIR * 128)

    W3c = W3[at][..., 0]  # [128, 64]
    w3a = np.zeros((NPAIR, 128, 128), dtype=f32)
    for p in range(NPAIR):
        w3a[p, :H, 2 * p] = W3c[2 * p]
        w3a[p, H:, 2 * p + 1] = W3c[2 * p + 1]
    w3d = np.ascontiguousarray(w3a.transpose(1, 0, 2)).reshape(128, NPAIR * 128)

    in_map = {
        "gt": np.ascontiguousarray(gt).astype(mdt),
        "w1": w1d.astype(mdt),
        "w2": w2d.astype(mdt),
        "w3": w3d.astype(mdt),
        "b3d": np.ascontiguousarray(b3[at]).astype(f32),
    }
    if with_b2:
        b2c = b2[at]  # [128, 64]
        in_map["b2d"] = np.ascontiguousarray(
            np.concatenate([b2c[0::2].T, b2c[1::2].T], axis=0)
        ).astype(f32)
    return in_map


def kernel(g, W1, b1, W2, b2, W3, b3):
    from concourse.bass_utils import run_bass_kernel_spmd

    g = np.asarray(g, dtype=np.float32)
    W1 = np.asarray(W1, dtype=np.float32)
    b1 = np.asarray(b1, dtype=np.float32)
    W2 = np.asarray(W2, dtype=np.float32)
    b2 = np.asarray(b2, dtype=np.float32)
    W3 = np.asarray(W3, dtype=np.float32)
    b3 = np.asarray(b3, dtype=np.float32)

    with_b2 = bool(np.any(b2))
    if with_b2 not in _compiled:
        _compiled[with_b2] = _build(with_b2)
    nc = _compiled[with_b2]

    in_maps = [
        _prep_core(c, g, W1, b1, W2, b2, W3, b3, with_b2) for c in range(NCORES)
    ]
    res = run_bass_kernel_spmd(nc, in_maps, list(range(NCORES)))

    e = np.empty((S, A), dtype=np.float32)
    for c in range(NCORES):
        e[:, c * ACORE : (c + 1) * ACORE] = res.results[c]["eo"].T
    return e
